# revision 4
# baseline (speedup 1.0000x reference)
"""MultiSimilarity-style metric-learning loss on 8 Trainium2 NeuronCores.

Algorithm (reference semantics):
    sim = batch @ batch.T                       [B, B], B=4096, D=1024
    pos_mask = same-label & ~eye & (sim < 1-eps)
    neg_mask = ~same-label
    min_pos = min(sim | pos), max_neg = max(sim | neg)          per row
    neg_keep = neg & (sim > min_pos - margin)
    pos_keep = pos & (sim < max_neg + margin)
    loss_row = [any(pos_keep) & any(neg_keep)] *
               ( log1p(sum exp(-2(sim-b)) | pos_keep)/2
               + log1p(sum exp(40(sim-b)) | neg_keep)/40 ),  b = beta[label]
    out = sum(loss_row) / B

Distribution: data-parallel over rows. Core c owns rows [c*512, (c+1)*512)
and computes its [512, 4096] slice of sim; the scalar reduction is done on
host (8 tiny [128,4] partials).

Device trick: the one-hot label matmul accumulates -BIG*same into the sim
PSUM, so a single activation-exp per side applies the pos/neg mask via
fp32 underflow (exp(arg - 2*BIG) == 0 exactly), and both row-statistics
(min_pos / max_neg) become plain max-reductions in exp space.
"""

import sys
import types

import numpy as np

B, D, NCLS = 4096, 1024, 100
N_CORES = 8
RPC = B // N_CORES          # rows per core = 512
RB_N = RPC // 128           # row blocks per core = 4
CC_N = B // 512             # column chunks = 8
KT_N = D // 128             # contraction tiles = 8
POS_W, NEG_W, MARGIN = 2.0, 40.0, 0.1
BIG = 128.0                 # mask offset; 2*BIG drives exp to exact 0
TINY = 1e-30

_CACHE = {}


def _install_ntff_hook_stub():
    """Make `antenv.axon_hooks` importable so run_bass_kernel_spmd's trace
    path degrades gracefully (or works, if the axon .so supports it)."""
    if "antenv.axon_hooks" in sys.modules:
        return
    mod = types.ModuleType("antenv.axon_hooks")
    mod._hook = None

    def set_axon_ntff_profile_hook(h):
        mod._hook = h

    def get_axon_ntff_profile_hook():
        return mod._hook

    mod.set_axon_ntff_profile_hook = set_axon_ntff_profile_hook
    mod.get_axon_ntff_profile_hook = get_axon_ntff_profile_hook
    sys.modules["antenv.axon_hooks"] = mod
    try:
        import antenv

        antenv.axon_hooks = mod
    except ImportError:
        pass
    try:
        from trn_agent_boot.trn_boot import _ntff_profile_via_ctypes

        hook = _ntff_profile_via_ctypes("/opt/axon/libaxon_pjrt.so")
        if hook is not None:
            mod.set_axon_ntff_profile_hook(hook)
    except Exception:
        pass


def build_program(use_f32r=True):
    import concourse.bacc as bacc
    import concourse.tile as tile
    from concourse import mybir

    f32 = mybir.dt.float32
    mm_dt = mybir.dt.float32r if use_f32r else mybir.dt.float32
    Exp = mybir.ActivationFunctionType.Exp
    Ln = mybir.ActivationFunctionType.Ln
    X = mybir.AxisListType.X
    op = mybir.AluOpType

    nc = bacc.Bacc("TRN2", target_bir_lowering=False, debug=False)

    # p-major layouts: btp[p, kt*4096 + col] = batch[col, kt*128 + p]
    btp = nc.dram_tensor("btp", [128, KT_N * B], mm_dt, kind="ExternalInput")
    # bto[p, kt*512 + r] = batch[own_row r, kt*128 + p]
    bto = nc.dram_tensor("bto", [128, KT_N * RPC], mm_dt, kind="ExternalInput")
    # oh[cls, col] = 1 if labels[col] == cls else 0 (rows 100..127 zero)
    oh = nc.dram_tensor("oh", [128, B], mm_dt, kind="ExternalInput")
    # ohs[cls, r] = -BIG if labels_own[r] == cls else 0
    ohs = nc.dram_tensor("ohs", [128, RPC], mm_dt, kind="ExternalInput")
    # per-partition activation biases, column rb
    bp = nc.dram_tensor("bp", [128, RB_N], f32, kind="ExternalInput")
    bn = nc.dram_tensor("bn", [128, RB_N], f32, kind="ExternalInput")
    out = nc.dram_tensor("out", [128, RB_N], f32, kind="ExternalOutput")

    with tile.TileContext(nc) as tc:
        with (
            tc.tile_pool(name="const", bufs=1) as constp,
            tc.tile_pool(name="btstream", bufs=2) as btsp,
            tc.tile_pool(name="ohstream", bufs=2) as ohsp,
            tc.tile_pool(name="epool", bufs=1) as ep,
            tc.tile_pool(name="slots", bufs=1) as slp,
            tc.tile_pool(name="tiny", bufs=1) as tp,
            tc.tile_pool(name="scratch", bufs=2) as scp,
            tc.tile_pool(name="psum", bufs=8, space="PSUM") as psp,
        ):
            # resident constants
            t_bto = constp.tile([128, KT_N, RPC], mm_dt, tag="bto")
            nc.sync.dma_start(t_bto[:], bto.ap().rearrange("p (k r) -> p k r", k=KT_N))
            t_ohs = constp.tile([128, RPC], mm_dt, tag="ohs")
            nc.sync.dma_start(t_ohs[:], ohs[:, :])
            t_bp = constp.tile([128, RB_N], f32, tag="bp")
            nc.sync.dma_start(t_bp[:], bp[:, :])
            t_bn = constp.tile([128, RB_N], f32, tag="bn")
            nc.sync.dma_start(t_bn[:], bn[:, :])

            e_p = [ep.tile([128, B], f32, tag=f"e_p{rb}", name=f"e_p{rb}") for rb in range(RB_N)]
            e_n = [ep.tile([128, B], f32, tag=f"e_n{rb}", name=f"e_n{rb}") for rb in range(RB_N)]
            maxp_s = slp.tile([128, RB_N, CC_N], f32, tag="maxp")
            maxn_s = slp.tile([128, RB_N, CC_N], f32, tag="maxn")
            sump_s = slp.tile([128, RB_N, CC_N], f32, tag="sump")
            sumn_s = slp.tile([128, RB_N, CC_N], f32, tag="sumn")

            btp_v = btp.ap().rearrange("p (k c) -> p k c", k=KT_N)

            # ---- phase 1: sim+mask matmuls, exp, per-chunk maxes ----
            for cc in range(CC_N):
                t_bt = btsp.tile([128, KT_N, 512], mm_dt, tag="bt")
                nc.sync.dma_start(t_bt[:], btp_v[:, :, cc * 512:(cc + 1) * 512])
                t_oh = ohsp.tile([128, 512], mm_dt, tag="oh")
                nc.sync.dma_start(t_oh[:], oh[:, cc * 512:(cc + 1) * 512])
                for rb in range(RB_N):
                    ps = psp.tile([128, 512], f32, tag="ps")
                    for kt in range(KT_N):
                        nc.tensor.matmul(
                            ps[:],
                            lhsT=t_bto[:, kt, rb * 128:(rb + 1) * 128],
                            rhs=t_bt[:, kt, :],
                            start=(kt == 0),
                            stop=False,
                        )
                    nc.tensor.matmul(
                        ps[:],
                        lhsT=t_ohs[:, rb * 128:(rb + 1) * 128],
                        rhs=t_oh[:],
                        start=False,
                        stop=True,
                    )
                    cs = slice(cc * 512, (cc + 1) * 512)
                    nc.scalar.activation(
                        e_p[rb][:, cs], ps[:], Exp,
                        bias=t_bp[:, rb:rb + 1], scale=-POS_W,
                    )
                    nc.scalar.activation(
                        e_n[rb][:, cs], ps[:], Exp,
                        bias=t_bn[:, rb:rb + 1], scale=NEG_W,
                    )
                    nc.vector.reduce_max(
                        maxp_s[:, rb, cc:cc + 1], e_p[rb][:, cs], axis=X)
                    nc.vector.reduce_max(
                        maxn_s[:, rb, cc:cc + 1], e_n[rb][:, cs], axis=X)

            # ---- phase 1.5: thresholds in exp space ----
            # maxep = exp(-2(min_pos - b)); maxen = exp(40(max_neg - b))
            # pos keep: e_p > exp(-0.2) * maxen^(-1/20)
            # neg keep: e_n > exp(-4) * maxep^(-20)   (clamped to exp(80))
            thrp, thrn = [], []
            for rb in range(RB_N):
                mep = tp.tile([128, 1], f32, tag=f"mep{rb}")
                nc.vector.reduce_max(mep[:], maxp_s[:, rb, :], axis=X)
                men = tp.tile([128, 1], f32, tag=f"men{rb}")
                nc.vector.reduce_max(men[:], maxn_s[:, rb, :], axis=X)
                nc.vector.tensor_scalar_max(mep[:], mep[:], TINY)
                nc.vector.tensor_scalar_max(men[:], men[:], TINY)
                lep = tp.tile([128, 1], f32, tag=f"lep{rb}")
                nc.scalar.activation(lep[:], mep[:], Ln)
                len_ = tp.tile([128, 1], f32, tag=f"len{rb}")
                nc.scalar.activation(len_[:], men[:], Ln)
                s0 = tp.tile([128, 1], f32, tag=f"s0{rb}")
                nc.vector.tensor_scalar(
                    s0[:], len_[:], -POS_W / NEG_W, -MARGIN * POS_W,
                    op0=op.mult, op1=op.add)
                tp_t = tp.tile([128, 1], f32, tag=f"thrp{rb}")
                nc.scalar.activation(tp_t[:], s0[:], Exp)
                s1 = tp.tile([128, 1], f32, tag=f"s1{rb}")
                nc.vector.tensor_scalar(
                    s1[:], lep[:], -NEG_W / POS_W, -MARGIN * NEG_W,
                    op0=op.mult, op1=op.add)
                nc.vector.tensor_scalar_min(s1[:], s1[:], 80.0)
                tn_t = tp.tile([128, 1], f32, tag=f"thrn{rb}")
                nc.scalar.activation(tn_t[:], s1[:], Exp)
                thrp.append(tp_t)
                thrn.append(tn_t)

            # ---- phase 2: thresholded sums ----
            for cc in range(CC_N):
                for rb in range(RB_N):
                    cs = slice(cc * 512, (cc + 1) * 512)
                    sc1 = scp.tile([128, 512], f32, tag="sc")
                    nc.vector.scalar_tensor_tensor(
                        sc1[:], e_p[rb][:, cs], thrp[rb][:], e_p[rb][:, cs],
                        op0=op.is_gt, op1=op.mult,
                        accum_out=sump_s[:, rb, cc:cc + 1])
                    sc2 = scp.tile([128, 512], f32, tag="sc")
                    nc.vector.scalar_tensor_tensor(
                        sc2[:], e_n[rb][:, cs], thrn[rb][:], e_n[rb][:, cs],
                        op0=op.is_gt, op1=op.mult,
                        accum_out=sumn_s[:, rb, cc:cc + 1])

            # ---- phase 3: per-row losses ----
            t_out = tp.tile([128, RB_N], f32, tag="out")
            for rb in range(RB_N):
                pos_sum = tp.tile([128, 1], f32, tag=f"possum{rb}")
                nc.vector.reduce_sum(pos_sum[:], sump_s[:, rb, :], axis=X)
                neg_sum = tp.tile([128, 1], f32, tag=f"negsum{rb}")
                nc.vector.reduce_sum(neg_sum[:], sumn_s[:, rb, :], axis=X)
                l1 = tp.tile([128, 1], f32, tag=f"l1{rb}")
                nc.scalar.activation(l1[:], pos_sum[:], Ln, bias=1.0)
                l2 = tp.tile([128, 1], f32, tag=f"l2{rb}")
                nc.scalar.activation(l2[:], neg_sum[:], Ln, bias=1.0)
                v1 = tp.tile([128, 1], f32, tag=f"v1{rb}")
                nc.vector.tensor_scalar(v1[:], pos_sum[:], 0.0, None, op0=op.is_gt)
                v2 = tp.tile([128, 1], f32, tag=f"v2{rb}")
                nc.vector.tensor_scalar(v2[:], neg_sum[:], 0.0, None, op0=op.is_gt)
                nc.vector.tensor_mul(v1[:], v1[:], v2[:])
                l2s = tp.tile([128, 1], f32, tag=f"l2s{rb}")
                nc.vector.tensor_scalar_mul(l2s[:], l2[:], 1.0 / NEG_W)
                w = tp.tile([128, 1], f32, tag=f"w{rb}")
                nc.vector.scalar_tensor_tensor(
                    w[:], l1[:], 1.0 / POS_W, l2s[:], op0=op.mult, op1=op.add)
                nc.vector.tensor_mul(t_out[:, rb:rb + 1], w[:], v1[:])
            nc.sync.dma_start(out[:, :], t_out[:])

    nc.compile()
    return nc


def _host_inputs(batch, beta, labels):
    """Build per-core input maps (host-side data prep, all O(B*D) cheap)."""
    batch = np.ascontiguousarray(batch, dtype=np.float32)
    beta = np.asarray(beta, dtype=np.float32)
    labels = np.asarray(labels).astype(np.int64)

    bt = batch.T  # [D, B]
    # btp[p, kt*B + col] = bt[kt*128 + p, col]
    btp = np.ascontiguousarray(
        bt.reshape(KT_N, 128, B).transpose(1, 0, 2).reshape(128, KT_N * B))
    oh_all = np.zeros((128, B), dtype=np.float32)
    oh_all[labels, np.arange(B)] = 1.0
    b_all = beta[labels]  # [B]

    in_maps = []
    for c in range(N_CORES):
        rows = slice(c * RPC, (c + 1) * RPC)
        bto_c = np.ascontiguousarray(
            batch[rows].T.reshape(KT_N, 128, RPC)
            .transpose(1, 0, 2).reshape(128, KT_N * RPC))
        ohs_c = np.zeros((128, RPC), dtype=np.float32)
        ohs_c[labels[rows], np.arange(RPC)] = -BIG
        b_own = b_all[rows].reshape(RB_N, 128).T  # [128, RB_N] (part, rb)
        bp_c = np.ascontiguousarray(POS_W * (b_own - BIG), dtype=np.float32)
        bn_c = np.ascontiguousarray(-NEG_W * b_own, dtype=np.float32)
        in_maps.append({
            "btp": btp, "bto": bto_c, "oh": oh_all,
            "ohs": ohs_c, "bp": bp_c, "bn": bn_c,
        })
    return in_maps


def run_on_device(batch, beta, labels, trace=False, use_f32r=True):
    _install_ntff_hook_stub()
    from concourse.bass_utils import run_bass_kernel_spmd

    key = ("nc", use_f32r)
    if key not in _CACHE:
        _CACHE[key] = build_program(use_f32r=use_f32r)
    nc = _CACHE[key]
    in_maps = _host_inputs(batch, beta, labels)
    res = run_bass_kernel_spmd(nc, in_maps, list(range(N_CORES)), trace=trace)
    per_row = np.stack([res.results[c]["out"] for c in range(N_CORES)])
    # per_row[c, p, rb] is the loss of global row c*512 + rb*128 + p
    total = np.float32(per_row.astype(np.float64).sum() / B)
    return total, res


def kernel(batch, beta, labels):
    total, _ = run_on_device(batch, beta, labels, trace=False)
    return np.float32(total)
